# revision 1
# baseline (speedup 1.0000x reference)
"""LocallyConnected2d (3x3, pad 1) Trainium2 kernel.

Problem: out[b,o,h,w] = sum_{c,k} x_pad[b,c,h+k//3,w+k%3] * W[o,c,h,w,k]
  x: [16, 64, 56, 56] f32, W: [1, 64, 64, 56, 56, 9] f32 -> out [16, 64, 56, 56] f32

Strategy (8 cores, H sharded, 7 rows/core):
  The weight (462 MB) is used exactly once per element -> memory bound.
  Host pre-permutes the weight into per-core contiguous SBUF slab layout so the
  device streams it with full-width [128, N] contiguous DMAs.

  Per output location (h, w): out_loc[o, b] = sum_{c,k} W[ck, o] * xpatch[ck, b]
  done as 5 fp32 matmuls accumulating into one PSUM tile [64(o), 16(b)]:
    - 4 matmuls with K=128: two 3x3-taps stacked on the partition dim.
      The upper 64 partitions hold shifted copies of x so that a single AP
      (common free offset across partitions) reads tap k on the lower half and
      tap k' on the upper half: tap pairs {0,3},{1,4},{2,5} need offset delta
      58 (one padded row), pair {6,7} needs delta 1.
    - 1 matmul with K=64 for tap 8, alternating partition halves by w parity so
      each half carries exactly 9 taps per w-pair (keeps the weight slab dense).
  The weight is the *stationary* operand: fp32 moving operands stream at 4
  cycles/row, while LDWEIGHTS loads 1 column/cycle, so the big operand must
  ride the weight-load path (x streams as the 16-wide moving operand).
"""

import numpy as np

B, C, O, H, W = 16, 64, 64, 56, 56
NCORES = 8
HPC = H // NCORES          # 7 output rows per core
XROWS = HPC + 2            # 9 padded-x rows per core
XW = W + 2                 # 58
BLK = XROWS * XW           # 522 floats per (b, c) x block
XCOPY = B * BLK            # 8352 floats per x copy per partition
SPR = 2                    # weight slabs per output row
NSLAB = HPC * SPR          # 14 weight slabs per core
WSLAB = W // SPR           # 28 w positions per slab
WPS = WSLAB // 2           # 14 w-pairs per slab -> 14*9*64 floats/partition

# slot mapping within a w-pair (9 slots of 64 floats per partition):
#   even w: slots 0-3 = chunks 0-3, slot 4 = tap8 (lower half: even w, upper: odd w)
#   odd  w: slots 5-8 = chunks 0-3
K_LO = [0, 1, 2, 6]        # lower-half tap per chunk 0-3
K_HI = [3, 4, 5, 7]        # upper-half tap per chunk 0-3

_CACHE = {}


def _host_prep(x, weight):
    """Build per-core device input arrays (layout transforms, host-side only)."""
    x = np.ascontiguousarray(x, dtype=np.float32)
    w0 = weight.reshape(O, C, H, W, 9).astype(np.float32, copy=False)

    xpad = np.zeros((B, C, H + 2, W + 2), np.float32)
    xpad[:, :, 1:-1, 1:-1] = x

    xs_list, ws_list = [], []
    for core in range(NCORES):
        h0 = core * HPC
        # x copies: [128, 2*XCOPY]; lower 64 partitions (c) = [plain, plain],
        # upper = [shift-by-1, shift-by-58]
        xc = xpad[:, :, h0:h0 + XROWS, :]                     # [B, C, 9, 58]
        plain = np.ascontiguousarray(xc.transpose(1, 0, 2, 3)).reshape(C, XCOPY)
        sh1 = np.zeros_like(plain)
        sh1[:, :-1] = plain[:, 1:]
        sh58 = np.zeros_like(plain)
        sh58[:, :-58] = plain[:, 58:]
        xdev = np.empty((128, 2 * XCOPY), np.float32)
        xdev[:64, :XCOPY] = plain
        xdev[:64, XCOPY:] = plain
        xdev[64:, :XCOPY] = sh1
        xdev[64:, XCOPY:] = sh58
        xs_list.append(xdev.reshape(128, 2 * B, BLK))

        # weight slabs: S[h, p=(s,c), wp, slot, o]
        wc = w0[:, :, h0:h0 + HPC, :, :]                       # [O, C, 7, 56, 9]
        wt = wc.transpose(2, 1, 3, 4, 0)                       # [7, C, 56, 9, O]
        we = wt[:, :, 0::2]                                    # [7, C, 28, 9, O] even w
        wo = wt[:, :, 1::2]
        S = np.empty((HPC, 128, W // 2, 9, O), np.float32)
        S[:, :64, :, 0:4] = we[:, :, :, K_LO, :]
        S[:, :64, :, 4] = we[:, :, :, 8, :]
        S[:, :64, :, 5:9] = wo[:, :, :, K_LO, :]
        S[:, 64:, :, 0:4] = we[:, :, :, K_HI, :]
        S[:, 64:, :, 4] = wo[:, :, :, 8, :]
        S[:, 64:, :, 5:9] = wo[:, :, :, K_HI, :]
        # split each row into SPR slabs of WPS w-pairs
        Sr = S.reshape(HPC, 128, SPR, WPS, 9, O).transpose(0, 2, 1, 3, 4, 5)
        ws_list.append(np.ascontiguousarray(Sr).reshape(NSLAB, 128, WPS * 9, O))
    return xs_list, ws_list


def _build_program(repeat=1, mode="full", mmdt="f32"):
    import concourse.mybir as mybir
    import concourse.tile as tile
    from concourse import bacc

    f32 = mybir.dt.float32
    mdt = f32 if mmdt == "f32" else mybir.dt.bfloat16
    nc = bacc.Bacc("TRN2", target_bir_lowering=False, debug=False,
                   num_devices=NCORES)
    xs = nc.dram_tensor("xs", [128, 2 * B, BLK], mdt, kind="ExternalInput")
    ws = nc.dram_tensor("ws", [NSLAB, 128, WPS * 9, O], mdt, kind="ExternalInput")
    out = nc.dram_tensor("out", [HPC, O, W, B], f32, kind="ExternalOutput")

    with tile.TileContext(nc) as tc:
        with tc.tile_pool(name="xp", bufs=1) as xpool, \
             tc.tile_pool(name="wp", bufs=3) as wpool, \
             tc.tile_pool(name="op", bufs=2) as opool, \
             tc.tile_pool(name="pp", bufs=8, space="PSUM") as ppool:

            def body(_iv=None):
                xt = xpool.tile([128, 2 * B, BLK], mdt, name="xt")
                nc.sync.dma_start(xt[:], xs[:])
                wt0 = None
                if mode == "pe":
                    wt0 = wpool.tile([128, WPS * 9, O], mdt, name="wt")
                    nc.sync.dma_start(wt0[:], ws[0])
                for h in range(HPC):
                    ot = opool.tile([O, W, B], f32, name="ot")
                    if mode == "dma":
                        nc.vector.memset(ot[:], 0.0)
                    for sub in range(SPR):
                        slab = h * SPR + sub
                        if mode == "pe":
                            wt = wt0
                        else:
                            wt = wpool.tile([128, WPS * 9, O], mdt, name="wt")
                            nc.sync.dma_start(wt[:], ws[slab])
                        if mode == "dma":
                            continue
                        for wl in range(WSLAB):
                            w = sub * WSLAB + wl
                            wp, par = wl // 2, w % 2
                            base = 5 * par
                            ps = ppool.tile([O, B], f32, name="ps")
                            # chunks 0-2: taps {t, t+3}, K=128, region 1 (delta 58)
                            for t in range(3):
                                q = wp * 9 + base + t
                                F = h * XW + w + t
                                nc.tensor.matmul(
                                    ps[:, :], wt[:, q:q + 1, :],
                                    xt[:, B:2 * B, F:F + 1],
                                    start=(t == 0), stop=False)
                            # chunk 3: taps {6,7}, K=128, region 0 (delta 1)
                            q = wp * 9 + base + 3
                            F = (h + 2) * XW + w
                            nc.tensor.matmul(
                                ps[:, :], wt[:, q:q + 1, :], xt[:, 0:B, F:F + 1],
                                start=False, stop=False)
                            # chunk 4: tap 8, K=64, half picked by w parity
                            q = wp * 9 + 4
                            F = (h + 2) * XW + w + 2
                            if par == 0:
                                nc.tensor.matmul(
                                    ps[:, :], wt[0:64, q:q + 1, :],
                                    xt[0:64, 0:B, F:F + 1],
                                    start=False, stop=True)
                            else:
                                nc.tensor.matmul(
                                    ps[:, :], wt[64:128, q:q + 1, :],
                                    xt[64:128, 0:B, F - 1:F],
                                    start=False, stop=True)
                            nc.vector.tensor_copy(ot[:, w:w + 1, :], ps[:, :])
                    nc.sync.dma_start(out[h], ot[:])

            if repeat > 1:
                with tc.For_i(0, repeat, 1):
                    body()
            else:
                body()
    nc.compile()
    return nc


def _get_program(repeat=1, mode="full", mmdt="f32"):
    key = ("nc", repeat, mode, mmdt)
    if key not in _CACHE:
        _CACHE[key] = _build_program(repeat, mode, mmdt)
    return _CACHE[key]


def run(x, weight, trace=False, repeat=1):
    from concourse.bass_utils import run_bass_kernel_spmd

    nc = _get_program(repeat)
    xs_list, ws_list = _host_prep(np.asarray(x), np.asarray(weight))
    in_maps = [{"xs": xs_list[i], "ws": ws_list[i]} for i in range(NCORES)]
    res = run_bass_kernel_spmd(nc, in_maps, core_ids=list(range(NCORES)),
                               trace=trace)
    parts = []
    for i in range(NCORES):
        oc = np.asarray(res.results[i]["out"])       # [HPC, O, W, B]
        parts.append(oc.transpose(3, 1, 0, 2))       # [B, O, HPC, W]
    full = np.concatenate(parts, axis=2)             # [B, O, H, W]
    return np.ascontiguousarray(full), res


def kernel(x, weight):
    out, _ = run(x, weight, trace=False)
    return out



# revision 2
# speedup vs baseline: 2.4216x; 2.4216x over previous
"""LocallyConnected2d (3x3, pad 1) Trainium2 kernel.

Problem: out[b,o,h,w] = sum_{c,k} x_pad[b,c,h+k//3,w+k%3] * W[o,c,h,w,k]
  x: [16, 64, 56, 56] f32, W: [1, 64, 64, 56, 56, 9] f32 -> out [16, 64, 56, 56] f32

Strategy (8 cores, H sharded, 7 rows/core):
  The weight (462 MB fp32) is used exactly once per element -> memory bound.
  Everything is cast to bf16 on the host (output error ~1e-3, well under the
  2e-2 gate), halving HBM traffic to ~35 MB/core (~97 us at 358 GB/s).

  Per output location (h, w): out_loc[b, o] = sum_{c,k} xpatch[ck, b] * W[ck, o]
  done as 5 bf16 matmuls accumulating into a PSUM slice [16(b), 64(o)]:
    - x patch rides the LDWEIGHTS (stationary) path: only 16 columns to load
      (~13 ns) vs 64 for the weight.
    - the per-location weight [128, 64] rides the moving path: 64 columns at
      1 col/cycle bf16 (~27 ns/MM), which is the fastest way to ingest the
      big operand. (fp32 did the opposite, because fp32 moving runs at 1/4
      rate; it also split every matmul into HI/LO pairs.)
  Chunks: 4 matmuls with K=128 (two 3x3-taps stacked on the partition dim via
  host-shifted x copies: tap pairs {0,3},{1,4},{2,5} use the +58-shift copy,
  {6,7} the +1-shift copy), plus 1 matmul with K=64 for tap 8 (partition half
  picked by w parity so the weight slab stays dense).

  7 locations accumulate into one PSUM bank [16, 448] -> one DVE copy per
  group -> one output-row DMA [16, 56*64] f32.
"""

import numpy as np

B, C, O, H, W = 16, 64, 64, 56, 56
NCORES = 8
HPC = H // NCORES          # 7 output rows per core
XROWS = HPC + 2            # 9 padded-x rows per core
XW = W + 2                 # 58
BLK = XROWS * XW           # 522 floats per (b, c) x block
XCOPY = B * BLK            # 8352 floats per x copy per partition
SPR = 2                    # weight slabs per output row
NSLAB = HPC * SPR          # 14 weight slabs per core
WSLAB = W // SPR           # 28 w positions per slab
WPS = WSLAB // 2           # 14 w-pairs per slab -> 14*9*64 elems/partition
GRP = 7                    # locations per PSUM accumulation group
NGS = WSLAB // GRP         # 4 groups per slab

# slot mapping within a w-pair (9 slots of 64 per partition):
#   even w: slots 0-3 = chunks 0-3, slot 4 = tap8 (lower half: even w, upper: odd w)
#   odd  w: slots 5-8 = chunks 0-3
K_LO = [0, 1, 2, 6]        # lower-half tap per chunk 0-3
K_HI = [3, 4, 5, 7]        # upper-half tap per chunk 0-3

_CACHE = {}


def _bf16(a):
    import ml_dtypes
    return a.astype(ml_dtypes.bfloat16)


def _host_prep(x, weight):
    """Build per-core device input arrays (layout transforms, host-side only)."""
    x = np.ascontiguousarray(x, dtype=np.float32)
    w0 = weight.reshape(O, C, H, W, 9).astype(np.float32, copy=False)

    xpad = np.zeros((B, C, H + 2, W + 2), np.float32)
    xpad[:, :, 1:-1, 1:-1] = x

    xs_list, ws_list = [], []
    for core in range(NCORES):
        h0 = core * HPC
        # x copies: [128, 2*XCOPY]; lower 64 partitions (c) = [plain, plain],
        # upper = [shift-by-1, shift-by-58]
        xc = xpad[:, :, h0:h0 + XROWS, :]                     # [B, C, 9, 58]
        plain = np.ascontiguousarray(xc.transpose(1, 0, 2, 3)).reshape(C, XCOPY)
        sh1 = np.zeros_like(plain)
        sh1[:, :-1] = plain[:, 1:]
        sh58 = np.zeros_like(plain)
        sh58[:, :-58] = plain[:, 58:]
        xdev = np.empty((128, 2 * XCOPY), np.float32)
        xdev[:64, :XCOPY] = plain
        xdev[:64, XCOPY:] = plain
        xdev[64:, :XCOPY] = sh1
        xdev[64:, XCOPY:] = sh58
        xs_list.append(_bf16(xdev.reshape(128, 2 * B, BLK)))

        # weight slabs: S[h, p=(s,c), wp, slot, o]
        wc = w0[:, :, h0:h0 + HPC, :, :]                       # [O, C, 7, 56, 9]
        wt = wc.transpose(2, 1, 3, 4, 0)                       # [7, C, 56, 9, O]
        we = wt[:, :, 0::2]                                    # [7, C, 28, 9, O] even w
        wo = wt[:, :, 1::2]
        S = np.empty((HPC, 128, W // 2, 9, O), np.float32)
        S[:, :64, :, 0:4] = we[:, :, :, K_LO, :]
        S[:, :64, :, 4] = we[:, :, :, 8, :]
        S[:, :64, :, 5:9] = wo[:, :, :, K_LO, :]
        S[:, 64:, :, 0:4] = we[:, :, :, K_HI, :]
        S[:, 64:, :, 4] = wo[:, :, :, 8, :]
        S[:, 64:, :, 5:9] = wo[:, :, :, K_HI, :]
        # split each row into SPR slabs of WPS w-pairs
        Sr = S.reshape(HPC, 128, SPR, WPS, 9, O).transpose(0, 2, 1, 3, 4, 5)
        ws_list.append(_bf16(np.ascontiguousarray(Sr).reshape(NSLAB, 128, WPS * 9, O)))
    return xs_list, ws_list


def _build_program(mode="full"):
    import concourse.mybir as mybir
    import concourse.tile as tile
    from concourse import bacc

    f32 = mybir.dt.float32
    bf16 = mybir.dt.bfloat16
    nc = bacc.Bacc("TRN2", target_bir_lowering=False, debug=False,
                   num_devices=NCORES)
    xs = nc.dram_tensor("xs", [128, 2 * B, BLK], bf16, kind="ExternalInput")
    ws = nc.dram_tensor("ws", [NSLAB, 128, WPS * 9, O], bf16, kind="ExternalInput")
    out = nc.dram_tensor("out", [HPC, B, W * O], f32, kind="ExternalOutput")

    with tile.TileContext(nc) as tc:
        with tc.tile_pool(name="xp", bufs=1) as xpool, \
             tc.tile_pool(name="wp", bufs=4) as wpool, \
             tc.tile_pool(name="op", bufs=2) as opool, \
             tc.tile_pool(name="pp", bufs=4, space="PSUM") as ppool:

            xt = xpool.tile([128, 2 * B, BLK], bf16, name="xt")
            nc.scalar.dma_start(xt[:], xs[:])
            wt0 = None
            if mode == "pe":
                wt0 = wpool.tile([128, WPS * 9, O], bf16, name="wt")
                nc.sync.dma_start(wt0[:], ws[0])
            for h in range(HPC):
                ot = opool.tile([B, W * O], f32, name="ot")
                if mode == "dma":
                    nc.vector.memset(ot[:], 0.0)
                for sub in range(SPR):
                    slab = h * SPR + sub
                    if mode == "pe":
                        wt = wt0
                    else:
                        wt = wpool.tile([128, WPS * 9, O], bf16, name="wt")
                        nc.sync.dma_start(wt[:], ws[slab])
                    if mode == "dma":
                        continue
                    for g in range(NGS):
                        ps = ppool.tile([B, GRP * O], f32, name="ps")
                        for l in range(GRP):
                            wl = g * GRP + l        # w within slab
                            w = sub * WSLAB + wl    # w within row
                            wp, par = wl // 2, wl % 2
                            base = 5 * par
                            po = ps[:, l * O:(l + 1) * O]
                            # chunks 0-2: taps {t, t+3}, K=128, +58-shift copy
                            for t in range(3):
                                q = wp * 9 + base + t
                                F = h * XW + w + t
                                nc.tensor.matmul(
                                    po, xt[:, B:2 * B, F:F + 1],
                                    wt[:, q:q + 1, :],
                                    start=(t == 0), stop=False)
                            # chunk 3: taps {6,7}, K=128, +1-shift copy
                            q = wp * 9 + base + 3
                            F = (h + 2) * XW + w
                            nc.tensor.matmul(
                                po, xt[:, 0:B, F:F + 1], wt[:, q:q + 1, :],
                                start=False, stop=False)
                            # chunk 4: tap 8, K=64, half picked by w parity
                            q = wp * 9 + 4
                            if par == 0:
                                F = (h + 2) * XW + w + 2
                                nc.tensor.matmul(
                                    po, xt[0:64, 0:B, F:F + 1],
                                    wt[0:64, q:q + 1, :],
                                    start=False, stop=True)
                            else:
                                F = (h + 2) * XW + w + 1
                                nc.tensor.matmul(
                                    po, xt[64:128, 0:B, F:F + 1],
                                    wt[64:128, q:q + 1, :],
                                    start=False, stop=True)
                        wbase = (sub * WSLAB + g * GRP) * O
                        nc.vector.tensor_copy(ot[:, wbase:wbase + GRP * O],
                                              ps[:, :])
                nc.sync.dma_start(out[h], ot[:])
    nc.compile()
    return nc


def _get_program(mode="full"):
    key = ("nc", mode)
    if key not in _CACHE:
        _CACHE[key] = _build_program(mode)
    return _CACHE[key]


def run(x, weight, trace=False, mode="full"):
    from concourse.bass_utils import run_bass_kernel_spmd

    nc = _get_program(mode)
    xs_list, ws_list = _host_prep(np.asarray(x), np.asarray(weight))
    in_maps = [{"xs": xs_list[i], "ws": ws_list[i]} for i in range(NCORES)]
    res = run_bass_kernel_spmd(nc, in_maps, core_ids=list(range(NCORES)),
                               trace=trace)
    parts = []
    for i in range(NCORES):
        oc = np.asarray(res.results[i]["out"])       # [HPC, B, W*O]
        oc = oc.reshape(HPC, B, W, O)
        parts.append(oc.transpose(1, 3, 0, 2))       # [B, O, HPC, W]
    full = np.concatenate(parts, axis=2)             # [B, O, H, W]
    return np.ascontiguousarray(full), res


def kernel(x, weight):
    out, _ = run(x, weight, trace=False)
    return out


# revision 18
# speedup vs baseline: 4.2346x; 1.7487x over previous
"""LocallyConnected2d (3x3, pad 1) Trainium2 kernel.

Problem: out[b,o,h,w] = sum_{c,k} x_pad[b,c,h+k//3,w+k%3] * W[o,c,h,w,k]
  x: [16, 64, 56, 56] f32, W: [1, 64, 64, 56, 56, 9] f32 -> out [16, 64, 56, 56] f32

Strategy (8 cores, H sharded, 7 rows/core, all bf16 on device):
  The weight is used exactly once per element -> memory bound. bf16 halves
  HBM traffic to ~35 MB/core (~97 us at 358 GB/s); output error ~3e-3 vs
  the 2e-2 gate.

  To amortize per-instruction PE overhead, 7 same-parity output locations
  are batched into ONE matmul per contraction chunk:
    stationary lhsT = x patches [K, 7*16]  (7 locations x 16 batch, l-major)
    moving rhs      = weights   [K, 7*64]  (those locations' weights)
    psum out        = [112, 448], of which only the 7 diagonal [16,64]
                      blocks (l==l') are wanted.
  The off-diagonal compute is free: the weight stream (1 col/cycle bf16,
  each weight element enters the PE exactly once) is the true floor, and
  this shape reaches it with only 5 matmuls per 7 locations (280 MMs/core
  vs 1960 unbatched). Diagonal blocks are extracted by small PSUM->SBUF
  copies alternating between the Vector and Scalar engines.

  Contraction chunks per location (K = c x taps, 576 total):
    chunks 0-2: tap pairs {t, t+3}, K=128, via the +58-shifted x copy
    chunk  3:   taps {6,7},         K=128, via the +1-shifted x copy
    chunk  4:   tap 8,              K=64, partition half picked by w parity
  Same-parity grouping (w = wbase, wbase+2, ..., wbase+12) keeps every
  chunk's x offsets and weight slots at a uniform stride so each group
  chunk is a single strided AP.
"""

import numpy as np

B, C, O, H, W = 16, 64, 64, 56, 56
NCORES = 8
HPC = H // NCORES          # 7 output rows per core
XROWS = HPC + 2            # 9 padded-x rows per core
XW = W + 2                 # 58
BLK = XROWS * XW           # 522 x elems per (copy, b) block per partition
SPR = 2                    # weight slabs per output row
NSLAB = HPC * SPR          # 14 weight slabs per core
WSLAB = W // SPR           # 28 w positions per slab
WPS = WSLAB // 2           # 14 w-pairs per slab
GRP = 7                    # locations per batched matmul group

# slot mapping within a w-pair (9 slots of 64 per partition):
#   even w: slots 0-3 = chunks 0-3, slot 4 = tap8 (lower half: even w, upper: odd w)
#   odd  w: slots 5-8 = chunks 0-3
K_LO = [0, 1, 2, 6]        # lower-half tap per chunk 0-3
K_HI = [3, 4, 5, 7]        # upper-half tap per chunk 0-3

_CACHE = {}


def _bf16(a):
    import ml_dtypes
    return a.astype(ml_dtypes.bfloat16)


def _host_prep(x, weight):
    """Build per-core device input arrays (layout transforms, host-side only)."""
    x = np.ascontiguousarray(x, dtype=np.float32)
    w0 = weight.reshape(O, C, H, W, 9).astype(np.float32, copy=False)

    xpad = np.zeros((B, C, H + 2, W + 2), np.float32)
    xpad[:, :, 1:-1, 1:-1] = x

    xs_list, ws_list = [], []
    for core in range(NCORES):
        h0 = core * HPC
        # x copies, F-major then b: [128, cpy, BLK, B]
        #   cpy 0: lower 64 partitions (c) = plain, upper = shift-by-1
        #   cpy 1: lower = plain, upper = shift-by-58
        xc = xpad[:, :, h0:h0 + XROWS, :]                     # [B, C, 9, 58]
        plain = np.ascontiguousarray(xc.transpose(1, 2, 3, 0)).reshape(C, BLK, B)
        sh1 = np.zeros_like(plain)
        sh1[:, :-1] = plain[:, 1:]
        sh58 = np.zeros_like(plain)
        sh58[:, :-58] = plain[:, 58:]
        xdev = np.empty((128, 2, BLK, B), np.float32)
        xdev[:64, 0] = plain
        xdev[:64, 1] = plain
        xdev[64:, 0] = sh1
        xdev[64:, 1] = sh58
        # de-interleave F parity: [128, cpy, F%2, F//2, b] so a group's
        # stationary patch (7 locations x 16 b, F stride 2) is contiguous
        x4 = xdev.reshape(128, 2, BLK // 2, 2, B).transpose(0, 1, 3, 2, 4)
        xs_list.append(_bf16(np.ascontiguousarray(x4)))

        # weight slabs: S[h, p=(s,c), wp, slot, o]
        wc = w0[:, :, h0:h0 + HPC, :, :]                       # [O, C, 7, 56, 9]
        wt = wc.transpose(2, 1, 3, 4, 0)                       # [7, C, 56, 9, O]
        we = wt[:, :, 0::2]                                    # [7, C, 28, 9, O] even w
        wo = wt[:, :, 1::2]
        S = np.empty((HPC, 128, W // 2, 9, O), np.float32)
        S[:, :64, :, 0:4] = we[:, :, :, K_LO, :]
        S[:, :64, :, 4] = we[:, :, :, 8, :]
        S[:, :64, :, 5:9] = wo[:, :, :, K_LO, :]
        S[:, 64:, :, 0:4] = we[:, :, :, K_HI, :]
        S[:, 64:, :, 4] = wo[:, :, :, 8, :]
        S[:, 64:, :, 5:9] = wo[:, :, :, K_HI, :]
        # split each row into SPR slabs of WPS w-pairs
        Sr = S.reshape(HPC, 128, SPR, WPS, 9, O).transpose(0, 2, 1, 3, 4, 5)
        Sr = np.ascontiguousarray(Sr).reshape(NSLAB, 128, 2, GRP, 9, O)
        # regroup to per-(group, chunk) contiguous [448] blocks:
        # block bi = wph*9 + {0-3: even chunks, 4-7: odd chunks, 8: tap8}
        SLOTMAP = [0, 1, 2, 3, 5, 6, 7, 8, 4]
        T = Sr[:, :, :, :, SLOTMAP, :].transpose(0, 1, 2, 4, 3, 5)
        ws_list.append(_bf16(np.ascontiguousarray(T).reshape(
            NSLAB, 128, 18, GRP * O)))
    return xs_list, ws_list


def _build_program(mode="full"):
    import concourse.mybir as mybir
    import concourse.tile as tile
    from concourse import bacc

    f32 = mybir.dt.float32
    bf16 = mybir.dt.bfloat16
    nc = bacc.Bacc("TRN2", target_bir_lowering=False, debug=False,
                   num_devices=NCORES)
    # x as [128, cpy, F%2, F/2, b]: fixed-parity F slices are contiguous
    xs = nc.dram_tensor("xs", [128, 2, 2, BLK // 2, B], bf16,
                        kind="ExternalInput")
    # weights as per-(group, chunk) contiguous [448] blocks
    ws = nc.dram_tensor("ws", [NSLAB, 128, 18, GRP * O], bf16,
                        kind="ExternalInput")
    # out row: partition l*16+b (l = location lane), free (group, pair-col);
    # location pairs are copied as [32, 128] blocks (half junk, 32-aligned
    # partition bases for the ACT quadrant rule); host strips the junk
    out = nc.dram_tensor("out", [HPC, GRP * B, 8 * 2 * O], f32,
                         kind="ExternalOutput")

    with tile.TileContext(nc) as tc:
        with tc.tile_pool(name="xp", bufs=1) as xpool, \
             tc.tile_pool(name="wp", bufs=4) as wpool, \
             tc.tile_pool(name="op", bufs=2) as opool, \
             tc.tile_pool(name="pp", bufs=4, space="PSUM") as ppool:

            xt = xpool.tile([128, 2, 2, BLK // 2, B], bf16, name="xt")
            nc.scalar.dma_start(xt[:], xs[:])

            def xap(lo, hi, cpy, F0):
                # [hi-lo, 7, B] x patch: partitions lo:hi, copy cpy, 7
                # locations starting at offset F0 with stride 2 (= one step
                # of the halved-F dim, in the F0%2 parity plane); (7, B) is
                # contiguous so it collapses to one 112-wide free dim
                return xt[lo:hi, cpy, F0 % 2, F0 // 2:F0 // 2 + GRP, :]

            ncopy = 0
            wt0 = None
            if mode == "pe":
                wt0 = wpool.tile([128, 18, GRP * O], bf16, name="wt")
                nc.sync.dma_start(wt0[:], ws[0])
            for h in range(HPC):
                ot = opool.tile([GRP * B, 8 * 2 * O], f32, name="ot")
                if mode == "dma":
                    nc.vector.memset(ot[:], 0.0)
                for sub in range(SPR):
                    slab = h * SPR + sub
                    if mode == "pe":
                        wt = wt0
                    else:
                        wt = wpool.tile([128, 18, GRP * O], bf16, name="wt")
                        nc.sync.dma_start(wt[:], ws[slab])
                    if mode == "dma":
                        continue
                    for g in range(4):
                        par = g % 2           # 0: even w group, 1: odd
                        wph = g // 2          # w-pair half (0-6 or 7-13)
                        ws0 = sub * WSLAB + 14 * wph + par  # first w in group
                        bi0 = wph * 9 + par * 4
                        ps = ppool.tile([GRP * B, GRP * O], f32, name="ps")
                        # chunks 0-2: taps {t, t+3}, K=128, +58-shift copy
                        for t in range(3):
                            nc.tensor.matmul(
                                ps[:, :], xap(0, 128, 1, h * XW + ws0 + t),
                                wt[:, bi0 + t, :],
                                start=(t == 0), stop=False)
                        # chunk 3: taps {6,7}, K=128, +1-shift copy
                        nc.tensor.matmul(
                            ps[:, :], xap(0, 128, 0, (h + 2) * XW + ws0),
                            wt[:, bi0 + 3, :],
                            start=False, stop=False)
                        # chunk 4: tap 8, K=64, half picked by parity
                        if par == 0:
                            nc.tensor.matmul(
                                ps[:, :], xap(0, 64, 0, (h + 2) * XW + ws0 + 2),
                                wt[0:64, wph * 9 + 8, :],
                                start=False, stop=True)
                        else:
                            nc.tensor.matmul(
                                ps[:, :], xap(64, 128, 0, (h + 2) * XW + ws0 + 1),
                                wt[64:128, wph * 9 + 8, :],
                                start=False, stop=True)
                        # extract the diagonal as 32-aligned blocks: three
                        # [32, 128] pair copies (50% junk cols) + one
                        # [16, 64] for location 6 at base 96
                        gi = sub * 4 + g
                        oc0 = gi * 2 * O
                        for a in range(3):
                            dst = ot[32 * a:32 * a + 32, oc0:oc0 + 2 * O]
                            src = ps[32 * a:32 * a + 32,
                                     2 * a * O:2 * a * O + 2 * O]
                            if ncopy % 2 == 0:
                                nc.vector.tensor_copy(dst, src)
                            else:
                                nc.scalar.copy(dst, src)
                            ncopy += 1
                        dst = ot[96:112, oc0:oc0 + O]
                        src = ps[96:112, 6 * O:7 * O]
                        if ncopy % 2 == 0:
                            nc.vector.tensor_copy(dst, src)
                        else:
                            nc.scalar.copy(dst, src)
                        ncopy += 1
                nc.sync.dma_start(out[h], ot[:])
    nc.compile()
    return nc


def _get_program(mode="full"):
    key = ("nc", mode)
    if key not in _CACHE:
        _CACHE[key] = _build_program(mode)
    return _CACHE[key]


def run(x, weight, trace=False, mode="full"):
    from concourse.bass_utils import run_bass_kernel_spmd

    nc = _get_program(mode)
    xs_list, ws_list = _host_prep(np.asarray(x), np.asarray(weight))
    in_maps = [{"xs": xs_list[i], "ws": ws_list[i]} for i in range(NCORES)]
    res = run_bass_kernel_spmd(nc, in_maps, core_ids=list(range(NCORES)),
                               trace=trace)
    full = np.empty((B, O, H, W), np.float32)
    for i in range(NCORES):
        oc = np.asarray(res.results[i]["out"])       # [HPC, GRP*B, 8*2*O]
        for gi in range(8):
            sub, g = divmod(gi, 4)
            ws0 = sub * WSLAB + 14 * (g // 2) + (g % 2)
            for l in range(GRP):
                a, r = divmod(l, 2)
                blk = oc[:, 32 * a + 16 * r:32 * a + 16 * r + B,
                         (2 * gi + r) * O:(2 * gi + r + 1) * O]
                # [h, b, o] -> [b, o, h]
                full[:, :, i * HPC:(i + 1) * HPC, ws0 + 2 * l] = \
                    blk.transpose(1, 2, 0)
    return full, res


def kernel(x, weight):
    out, _ = run(x, weight, trace=False)
    return out


# revision 22
# speedup vs baseline: 4.5556x; 1.0758x over previous
"""LocallyConnected2d (3x3, pad 1) Trainium2 kernel.

Problem: out[b,o,h,w] = sum_{c,k} x_pad[b,c,h+k//3,w+k%3] * W[o,c,h,w,k]
  x: [16, 64, 56, 56] f32, W: [1, 64, 64, 56, 56, 9] f32 -> out [16, 64, 56, 56] f32

Strategy (8 cores, H sharded, 7 rows/core, all bf16 on device):
  The weight is used exactly once per element -> memory bound. bf16 halves
  HBM traffic to ~35 MB/core (~97 us at 358 GB/s); output error ~3e-3 vs
  the 2e-2 gate.

  To amortize per-instruction PE overhead, 7 same-parity output locations
  are batched into ONE matmul per contraction chunk:
    stationary lhsT = x patches [K, 7*16]  (7 locations x 16 batch, l-major)
    moving rhs      = weights   [K, 7*64]  (those locations' weights)
    psum out        = [112, 448], of which only the 7 diagonal [16,64]
                      blocks (l==l') are wanted.
  The off-diagonal compute is free: the weight stream (1 col/cycle bf16,
  each weight element enters the PE exactly once) is the true floor, and
  this shape reaches it with only 5 matmuls per 7 locations (280 MMs/core
  vs 1960 unbatched). Diagonal blocks are extracted by small PSUM->SBUF
  copies alternating between the Vector and Scalar engines.

  Contraction chunks per location (K = c x taps, 576 total):
    chunks 0-2: tap pairs {t, t+3}, K=128, via the +58-shifted x copy
    chunk  3:   taps {6,7},         K=128, via the +1-shifted x copy
    chunk  4:   tap 8,              K=64, partition half picked by w parity
  Same-parity grouping (w = wbase, wbase+2, ..., wbase+12) keeps every
  chunk's x offsets and weight slots at a uniform stride so each group
  chunk is a single strided AP.
"""

import numpy as np

B, C, O, H, W = 16, 64, 64, 56, 56
NCORES = 8
HPC = H // NCORES          # 7 output rows per core
XROWS = HPC + 2            # 9 padded-x rows per core
XW = W + 2                 # 58
BLK = XROWS * XW           # 522 x elems per (copy, b) block per partition
SPR = 2                    # weight slabs per output row
NSLAB = HPC * SPR          # 14 weight slabs per core
WSLAB = W // SPR           # 28 w positions per slab
WPS = WSLAB // 2           # 14 w-pairs per slab
GRP = 7                    # locations per batched matmul group

# slot mapping within a w-pair (9 slots of 64 per partition):
#   even w: slots 0-3 = chunks 0-3, slot 4 = tap8 (lower half: even w, upper: odd w)
#   odd  w: slots 5-8 = chunks 0-3
K_LO = [0, 1, 2, 6]        # lower-half tap per chunk 0-3
K_HI = [3, 4, 5, 7]        # upper-half tap per chunk 0-3

_CACHE = {}


def _bf16(a):
    import ml_dtypes
    return a.astype(ml_dtypes.bfloat16)


def _host_prep(x, weight):
    """Build per-core device input arrays (layout transforms, host-side only)."""
    x = np.ascontiguousarray(x, dtype=np.float32)
    w0 = weight.reshape(O, C, H, W, 9).astype(np.float32, copy=False)

    xpad = np.zeros((B, C, H + 2, W + 2), np.float32)
    xpad[:, :, 1:-1, 1:-1] = x

    xs_list, ws_list = [], []
    for core in range(NCORES):
        h0 = core * HPC
        # x copies, F-major then b: [128, cpy, BLK, B]
        #   cpy 0: lower 64 partitions (c) = plain, upper = shift-by-1
        #   cpy 1: lower = plain, upper = shift-by-58
        xc = xpad[:, :, h0:h0 + XROWS, :]                     # [B, C, 9, 58]
        plain = np.ascontiguousarray(xc.transpose(1, 2, 3, 0)).reshape(C, BLK, B)
        sh1 = np.zeros_like(plain)
        sh1[:, :-1] = plain[:, 1:]
        sh58 = np.zeros_like(plain)
        sh58[:, :-58] = plain[:, 58:]
        xdev = np.empty((128, 2, BLK, B), np.float32)
        xdev[:64, 0] = plain
        xdev[:64, 1] = plain
        xdev[64:, 0] = sh1
        xdev[64:, 1] = sh58
        # de-interleave F parity: [128, cpy, F%2, F//2, b] so a group's
        # stationary patch (7 locations x 16 b, F stride 2) is contiguous
        x4 = xdev.reshape(128, 2, BLK // 2, 2, B).transpose(0, 1, 3, 2, 4)
        xs_list.append(_bf16(np.ascontiguousarray(x4)))

        # weight slabs: S[h, p=(s,c), wp, slot, o]
        wc = w0[:, :, h0:h0 + HPC, :, :]                       # [O, C, 7, 56, 9]
        wt = wc.transpose(2, 1, 3, 4, 0)                       # [7, C, 56, 9, O]
        we = wt[:, :, 0::2]                                    # [7, C, 28, 9, O] even w
        wo = wt[:, :, 1::2]
        S = np.empty((HPC, 128, W // 2, 9, O), np.float32)
        S[:, :64, :, 0:4] = we[:, :, :, K_LO, :]
        S[:, :64, :, 4] = we[:, :, :, 8, :]
        S[:, :64, :, 5:9] = wo[:, :, :, K_LO, :]
        S[:, 64:, :, 0:4] = we[:, :, :, K_HI, :]
        S[:, 64:, :, 4] = wo[:, :, :, 8, :]
        S[:, 64:, :, 5:9] = wo[:, :, :, K_HI, :]
        # split each row into SPR slabs of WPS w-pairs
        Sr = S.reshape(HPC, 128, SPR, WPS, 9, O).transpose(0, 2, 1, 3, 4, 5)
        Sr = np.ascontiguousarray(Sr).reshape(NSLAB, 128, 2, GRP, 9, O)
        # regroup to per-(group, chunk) contiguous [448] blocks, one
        # half-slab (w-pair half wph) per DMA quantum:
        # block bi = {0-3: even chunks, 4-7: odd chunks, 8: tap8}
        SLOTMAP = [0, 1, 2, 3, 5, 6, 7, 8, 4]
        T = Sr[:, :, :, :, SLOTMAP, :].transpose(0, 2, 1, 4, 3, 5)
        ws_list.append(_bf16(np.ascontiguousarray(T).reshape(
            NSLAB, 2, 128, 9, GRP * O)))
    return xs_list, ws_list


def _build_program(mode="full"):
    import concourse.mybir as mybir
    import concourse.tile as tile
    from concourse import bacc

    f32 = mybir.dt.float32
    bf16 = mybir.dt.bfloat16
    nc = bacc.Bacc("TRN2", target_bir_lowering=False, debug=False,
                   num_devices=NCORES)
    # x as [128, cpy, F%2, F/2, b]: fixed-parity F slices are contiguous
    xs = nc.dram_tensor("xs", [128, 2, 2, BLK // 2, B], bf16,
                        kind="ExternalInput")
    # weights as per-(group, chunk) contiguous [448] blocks, half-slab
    # (one wph) per DMA quantum
    ws = nc.dram_tensor("ws", [NSLAB, 2, 128, 9, GRP * O], bf16,
                        kind="ExternalInput")
    # out row: partition l*16+b (l = location lane), free (group, pair-col);
    # location pairs are copied as [32, 128] blocks (half junk, 32-aligned
    # partition bases for the ACT quadrant rule); host strips the junk.
    # bf16 to halve output traffic (quantization ~0.2%, gate is 2e-2).
    out = nc.dram_tensor("out", [HPC, GRP * B, 8 * 2 * O], bf16,
                         kind="ExternalOutput")

    with tile.TileContext(nc) as tc:
        with tc.tile_pool(name="xp", bufs=1) as xpool, \
             tc.tile_pool(name="wp", bufs=6) as wpool, \
             tc.tile_pool(name="op", bufs=2) as opool, \
             tc.tile_pool(name="pp", bufs=4, space="PSUM") as ppool:

            xt = xpool.tile([128, 2, 2, BLK // 2, B], bf16, name="xt")
            # the +58-shift copy (chunks 0-2) is needed first; the +1-shift
            # copy (chunks 3-4) is interleaved after the first half-slab
            nc.sync.dma_start(xt[:, 1], xs[:, 1])

            def xap(lo, hi, cpy, F0):
                # [hi-lo, 7, B] x patch: partitions lo:hi, copy cpy, 7
                # locations starting at offset F0 with stride 2 (= one step
                # of the halved-F dim, in the F0%2 parity plane); (7, B) is
                # contiguous so it collapses to one 112-wide free dim
                return xt[lo:hi, cpy, F0 % 2, F0 // 2:F0 // 2 + GRP, :]

            ncopy = 0
            nwdma = 0
            wt0 = None
            if mode == "pe":
                wt0 = wpool.tile([128, 9, GRP * O], bf16, name="wt")
                nc.sync.dma_start(wt0[:], ws[0, 0])
            for h in range(HPC):
                ot = opool.tile([GRP * B, 8 * 2 * O], bf16, name="ot")
                if mode == "dma":
                    nc.vector.memset(ot[:], 0.0)
                for sub in range(SPR):
                    slab = h * SPR + sub
                    for wph in range(2):
                        if mode == "pe":
                            wt = wt0
                        else:
                            wt = wpool.tile([128, 9, GRP * O], bf16,
                                            name="wt")
                            nc.sync.dma_start(wt[:], ws[slab, wph])
                            nwdma += 1
                            if nwdma == 1:
                                # second x copy right after the first
                                # half-slab so the pipeline starts early
                                nc.sync.dma_start(xt[:, 0], xs[:, 0])
                        if mode == "dma":
                            continue
                        for par in range(2):  # 0: even w group, 1: odd
                            ws0 = sub * WSLAB + 14 * wph + par
                            bi0 = par * 4
                            ps = ppool.tile([GRP * B, GRP * O], f32,
                                            name="ps")
                            # chunks 0-2: taps {t, t+3}, K=128, +58 copy
                            for t in range(3):
                                nc.tensor.matmul(
                                    ps[:, :],
                                    xap(0, 128, 1, h * XW + ws0 + t),
                                    wt[:, bi0 + t, :],
                                    start=(t == 0), stop=False)
                            # chunk 3: taps {6,7}, K=128, +1 copy
                            nc.tensor.matmul(
                                ps[:, :], xap(0, 128, 0, (h + 2) * XW + ws0),
                                wt[:, bi0 + 3, :],
                                start=False, stop=False)
                            # chunk 4: tap 8, K=64, half picked by parity
                            if par == 0:
                                nc.tensor.matmul(
                                    ps[:, :],
                                    xap(0, 64, 0, (h + 2) * XW + ws0 + 2),
                                    wt[0:64, 8, :],
                                    start=False, stop=True)
                            else:
                                nc.tensor.matmul(
                                    ps[:, :],
                                    xap(64, 128, 0, (h + 2) * XW + ws0 + 1),
                                    wt[64:128, 8, :],
                                    start=False, stop=True)
                            # extract the diagonal as 32-aligned blocks:
                            # three [32, 128] pair copies (50% junk cols)
                            # + one [16, 64] for location 6 at base 96
                            gi = sub * 4 + wph * 2 + par
                            oc0 = gi * 2 * O
                            for a in range(3):
                                dst = ot[32 * a:32 * a + 32,
                                         oc0:oc0 + 2 * O]
                                src = ps[32 * a:32 * a + 32,
                                         2 * a * O:2 * a * O + 2 * O]
                                if ncopy % 2 == 0:
                                    nc.vector.tensor_copy(dst, src)
                                else:
                                    nc.scalar.copy(dst, src)
                                ncopy += 1
                            dst = ot[96:112, oc0:oc0 + O]
                            src = ps[96:112, 6 * O:7 * O]
                            if ncopy % 2 == 0:
                                nc.vector.tensor_copy(dst, src)
                            else:
                                nc.scalar.copy(dst, src)
                            ncopy += 1
                nc.scalar.dma_start(out[h], ot[:])
    nc.compile()
    return nc


def _get_program(mode="full"):
    key = ("nc", mode)
    if key not in _CACHE:
        _CACHE[key] = _build_program(mode)
    return _CACHE[key]


def run(x, weight, trace=False, mode="full"):
    from concourse.bass_utils import run_bass_kernel_spmd

    nc = _get_program(mode)
    xs_list, ws_list = _host_prep(np.asarray(x), np.asarray(weight))
    in_maps = [{"xs": xs_list[i], "ws": ws_list[i]} for i in range(NCORES)]
    res = run_bass_kernel_spmd(nc, in_maps, core_ids=list(range(NCORES)),
                               trace=trace)
    full = np.empty((B, O, H, W), np.float32)
    for i in range(NCORES):
        oc = np.asarray(res.results[i]["out"])       # [HPC, GRP*B, 8*2*O]
        for gi in range(8):
            sub, g = divmod(gi, 4)
            ws0 = sub * WSLAB + 14 * (g // 2) + (g % 2)
            for l in range(GRP):
                a, r = divmod(l, 2)
                blk = oc[:, 32 * a + 16 * r:32 * a + 16 * r + B,
                         (2 * gi + r) * O:(2 * gi + r + 1) * O]
                # [h, b, o] -> [b, o, h]
                full[:, :, i * HPC:(i + 1) * HPC, ws0 + 2 * l] = \
                    blk.transpose(1, 2, 0)
    return full, res


def kernel(x, weight):
    out, _ = run(x, weight, trace=False)
    return out
